# revision 4
# baseline (speedup 1.0000x reference)
"""Trainium2 Bass kernel for nn_ArielDecoderCell1 (arithmetic-decoding LSTM cell).

Math summary (for the harness inputs: timeStep=20 > 0, tokens all PAD=0):
  - initial_softmax = LSTM-LM(PAD) = softmax(0) = uniform 1/V exactly
    (PAD is masked, h stays 0 — independent of the weights).
  - timeStep > 0  =>  unfolding_point := input_point, one_softmax := uniform.
  - decode on the uniform grid: cums k = (k+1)/4096 exactly in f32, so
    token = floor(p*4096) and new_coord = frac(p*4096), both bit-exact.
  - second LSTM over tokens[:, :timeStep+1]: steps 0..timeStep-1 are PAD
    (masked, h=c stay 0), so only ONE step runs from h=c=0:
        z = E[token] @ Wi + b      (h @ Wh == 0 exactly -> Wh unused,
                                    f-gate * c == 0 exactly -> f-gate unused)
        c = sigmoid(z_i) * tanh(z_g);  h = sigmoid(z_o) * tanh(c)
        one_softmax = softmax(h)   (rows with token==0 keep h=0 -> uniform)

Device strategy (8 cores): tensor-parallel over the hidden dim V=4096.
Core k owns hidden slice [k*512,(k+1)*512) and loads only the i/g/o gate
columns of Wi for its slice ([256,1536] = 1.5MB vs 64MB naive), computes its
h-shard + exp + local row-sum, AllGathers the 8 row-sum vectors (256B each),
normalizes and writes its 512-column shard of the softmax. The decode
(token/new_coord) is computed redundantly on every core (it is a few [64,1]
vector ops). sigmoid is computed as 0.5*tanh(0.5x)+0.5 so tanh+exp share one
ACT table set.
"""

import numpy as np

B, V, EMB, LAT, MAXLEN = 64, 4096, 256, 16, 64
PAD = 0
NCORES = 8
SH = V // NCORES  # 512 hidden units per core

# "mod": token = x - mod(x,1) via DVE mod ALU op (fast, needs HW mod support)
# "count": token = sum_k 1[k <= x] over an iota grid (slower, bulletproof)
TOKEN_MODE = "mod"
TRACE = False  # test.py can set kernel.TRACE=True to capture an NTFF profile
LAST_RESULTS = None  # BassKernelResults of the last device run (for test.py)

_prog_cache: dict = {}


# ----------------------------------------------------------------------------
# Device program
# ----------------------------------------------------------------------------
def _build_program(cd: int, ts: int, use_bias: bool, token_mode: str):
    from concourse import bacc, bass, mybir, tile
    from concourse.masks import make_identity

    f32 = mybir.dt.float32
    i32 = mybir.dt.int32
    AF = mybir.ActivationFunctionType
    OP = mybir.AluOpType

    nc = bacc.Bacc("TRN2", target_bir_lowering=False, debug=False,
                   num_devices=NCORES)

    ip_d = nc.dram_tensor("ip", [B, LAT], f32, kind="ExternalInput")
    tk_d = nc.dram_tensor("tk", [B, MAXLEN], f32, kind="ExternalInput")
    E_d = nc.dram_tensor("emb", [V, EMB], f32, kind="ExternalInput")
    krows = 257 if use_bias else 256
    wp_d = nc.dram_tensor("wp", [krows, 3 * SH], f32, kind="ExternalInput")

    out_d = nc.dram_tensor("out_shard", [B, SH], f32, kind="ExternalOutput")
    tko_d = nc.dram_tensor("tok_out", [B, MAXLEN], f32, kind="ExternalOutput")
    unf_d = nc.dram_tensor("unf_out", [B, LAT], f32, kind="ExternalOutput")

    with tile.TileContext(nc) as tc:
        with (
            tc.tile_pool(name="cpool", bufs=1) as cpool,
            tc.tile_pool(name="wpool", bufs=1) as wpool,
            tc.tile_pool(name="ppool", bufs=1, space="PSUM") as ppool,
            tc.tile_pool(name="dpool", bufs=1, space="DRAM") as dpool,
        ):
            # Prefetch the exp_and_others ACT table set (tanh+exp) while DMAs run.
            warm = cpool.tile([1, 8], f32, tag="warm")
            nc.vector.memset(warm[:], 0.0)
            nc.scalar.activation(warm[:], warm[:], AF.Tanh)

            # ---- weight shard load (2 x 768KB) ----
            w0 = wpool.tile([128, 3 * SH], f32, tag="w0")
            w1 = wpool.tile([128, 3 * SH], f32, tag="w1")
            nc.sync.dma_start(w0[:], wp_d[0:128, :])
            nc.sync.dma_start(w1[:], wp_d[128:256, :])
            if use_bias:
                br = wpool.tile([1, 3 * SH], f32, tag="br")
                nc.sync.dma_start(br[:], wp_d[256:257, :])
                ones1 = cpool.tile([1, B], f32, tag="ones1")
                nc.vector.memset(ones1[:], 1.0)

            # ---- decode: token = floor(p*4096), new_coord = frac(p*4096) ----
            ip_t = cpool.tile([B, LAT], f32, tag="ip")
            nc.sync.dma_start(ip_t[:], ip_d[:])
            xcol = cpool.tile([B, 1], f32, tag="xcol")
            nc.vector.tensor_scalar_mul(xcol[:], ip_t[:, cd:cd + 1], float(V))

            tokf = cpool.tile([B, 1], f32, tag="tokf")
            frac = cpool.tile([B, 1], f32, tag="frac")
            if token_mode == "mod":
                # floor(x) via int cast (any rounding mode) + fixup:
                #   r = round(x); r -= (r > x)
                toki_r = cpool.tile([B, 1], i32, tag="toki_r")
                nc.vector.tensor_copy(toki_r[:], xcol[:])
                tokr_f = cpool.tile([B, 1], f32, tag="tokr_f")
                nc.vector.tensor_copy(tokr_f[:], toki_r[:])
                too_big = cpool.tile([B, 1], f32, tag="too_big")
                nc.vector.tensor_scalar(out=too_big[:], in0=tokr_f[:],
                                        scalar1=xcol[:, :1], scalar2=None,
                                        op0=OP.is_gt)
                nc.vector.tensor_tensor(out=tokf[:], in0=tokr_f[:],
                                        in1=too_big[:], op=OP.subtract)
                nc.vector.tensor_tensor(out=frac[:], in0=xcol[:], in1=tokf[:],
                                        op=OP.subtract)
            else:
                # grid[j, n] = n+1 (same for every partition); count grid <= x
                grid_i = cpool.tile([B, V], i32, tag="grid_i")
                nc.gpsimd.iota(grid_i[:], pattern=[[1, V]], base=1,
                               channel_multiplier=0)
                grid_f = cpool.tile([B, V], f32, tag="grid_f")
                nc.vector.tensor_copy(grid_f[:], grid_i[:])
                cmp_t = cpool.tile([B, V], f32, tag="cmp")
                nc.vector.tensor_scalar(out=cmp_t[:], in0=grid_f[:],
                                        scalar1=xcol[:, :1], scalar2=None,
                                        op0=OP.is_le, accum_out=tokf[:])
                nc.vector.tensor_tensor(out=frac[:], in0=xcol[:], in1=tokf[:],
                                        op=OP.subtract)

            toki = cpool.tile([B, 1], i32, tag="toki")
            nc.vector.tensor_copy(toki[:], tokf[:])
            maskp = cpool.tile([B, 1], f32, tag="maskp")
            nc.vector.tensor_scalar(out=maskp[:], in0=tokf[:], scalar1=0.5,
                                    scalar2=None, op0=OP.is_ge)

            # ---- tokens / unfolding outputs ----
            tk_t = cpool.tile([B, MAXLEN], f32, tag="tk")
            nc.sync.dma_start(tk_t[:], tk_d[:])
            nc.vector.tensor_copy(tk_t[:, ts:ts + 1], tokf[:])
            nc.sync.dma_start(tko_d[:], tk_t[:])

            un_t = cpool.tile([B, LAT], f32, tag="un")
            nc.vector.tensor_copy(un_t[:], ip_t[:])
            nc.vector.tensor_copy(un_t[:, cd:cd + 1], frac[:])
            nc.sync.dma_start(unf_d[:], un_t[:])

            # ---- embedding gather x = E[token]  [B, EMB] ----
            x_t = cpool.tile([B, EMB], f32, tag="x")
            nc.gpsimd.indirect_dma_start(
                out=x_t[:], out_offset=None, in_=E_d[:],
                in_offset=bass.IndirectOffsetOnAxis(ap=toki[:, :1], axis=0))

            # ---- transpose x -> xT chunks [128, B] ----
            ident = cpool.tile([B, B], f32, tag="ident")
            make_identity(nc, ident[:])
            xT = []
            for c in range(2):
                tp = ppool.tile([128, B], f32, tag=f"tp{c}")
                nc.tensor.transpose(out=tp[:], in_=x_t[:, c * 128:(c + 1) * 128],
                                    identity=ident[:])
                xt_sb = cpool.tile([128, B], f32, tag=f"xT{c}")
                nc.vector.tensor_copy(xt_sb[:], tp[:])
                xT.append(xt_sb)

            # ---- gate matmuls + activations ----
            # gate blocks in wp: 0 -> i, 1 -> g, 2 -> o
            # sigmoid(z) = 0.5*tanh(0.5 z) + 0.5 (tanh & exp share a table set)
            acts = []
            for gi, pre in enumerate([0.5, 1.0, 0.5]):
                zg = ppool.tile([B, SH], f32, tag=f"z{gi}")
                cols = slice(gi * SH, (gi + 1) * SH)
                nc.tensor.matmul(zg[:], xT[0][:], w0[:, cols],
                                 start=True, stop=False)
                nc.tensor.matmul(zg[:], xT[1][:], w1[:, cols],
                                 start=False, stop=not use_bias)
                if use_bias:
                    nc.tensor.matmul(zg[:], ones1[:], br[:, cols],
                                     start=False, stop=True)
                a = cpool.tile([B, SH], f32, tag=f"a{gi}")
                nc.scalar.activation(a[:], zg[:], AF.Tanh, scale=pre)
                acts.append(a)

            i_t = cpool.tile([B, SH], f32, tag="i_t")
            nc.vector.tensor_scalar(out=i_t[:], in0=acts[0][:], scalar1=0.5,
                                    scalar2=0.5, op0=OP.mult, op1=OP.add)
            c_t = cpool.tile([B, SH], f32, tag="c_t")
            nc.vector.tensor_tensor(out=c_t[:], in0=i_t[:], in1=acts[1][:],
                                    op=OP.mult)
            tch = cpool.tile([B, SH], f32, tag="tch")
            nc.scalar.activation(tch[:], c_t[:], AF.Tanh)
            o_t = cpool.tile([B, SH], f32, tag="o_t")
            nc.vector.tensor_scalar(out=o_t[:], in0=acts[2][:], scalar1=0.5,
                                    scalar2=0.5, op0=OP.mult, op1=OP.add)
            h_t = cpool.tile([B, SH], f32, tag="h_t")
            nc.vector.tensor_tensor(out=h_t[:], in0=o_t[:], in1=tch[:],
                                    op=OP.mult)

            # e = exp(h * mask) with fused local row-sum
            e_t = cpool.tile([B, SH], f32, tag="e_t")
            sloc = cpool.tile([B, 1], f32, tag="sloc")
            nc.scalar.activation(e_t[:], h_t[:], AF.Exp, scale=maskp[:, :1],
                                 accum_out=sloc[:])

            # ---- AllGather the 8 local row-sum vectors ----
            cin = dpool.tile([B, 1], f32, tag="cin")
            nc.sync.dma_start(cin[:], sloc[:])
            cout = dpool.tile([NCORES * B, 1], f32, tag="cout")
            nc.gpsimd.collective_compute(
                "AllGather", mybir.AluOpType.bypass,
                replica_groups=[list(range(NCORES))],
                ins=[cin[:].opt()], outs=[cout[:].opt()])
            parts = cpool.tile([NCORES, B], f32, tag="parts")
            nc.sync.dma_start(parts[:],
                              cout[:].rearrange("(k b) o -> k (b o)", b=B))

            # S[b] = sum_k parts[k, b] via a tiny matmul with a ones vector
            ones8 = cpool.tile([NCORES, 1], f32, tag="ones8")
            nc.vector.memset(ones8[:], 1.0)
            S_ps = ppool.tile([B, 1], f32, tag="S")
            nc.tensor.matmul(S_ps[:], parts[:], ones8[:], start=True, stop=True)
            s_sb = cpool.tile([B, 1], f32, tag="s_sb")
            nc.vector.reciprocal(s_sb[:], S_ps[:])

            out_t = cpool.tile([B, SH], f32, tag="out_t")
            nc.vector.tensor_scalar_mul(out_t[:], e_t[:], s_sb[:, :1])
            nc.sync.dma_start(out_d[:], out_t[:])

    nc.compile()
    return nc


def _get_program(cd: int, ts: int, use_bias: bool):
    key = (cd, ts, use_bias, TOKEN_MODE)
    if key not in _prog_cache:
        _prog_cache[key] = _build_program(cd, ts, use_bias, TOKEN_MODE)
    return _prog_cache[key]


# ----------------------------------------------------------------------------
# Host fallback (mirrors reference.py in numpy; not used for the harness
# inputs, kept for general correctness)
# ----------------------------------------------------------------------------
def _softmax_np(x):
    m = np.max(x, axis=-1, keepdims=True)
    e = np.exp((x - m).astype(np.float32)).astype(np.float32)
    return (e / np.sum(e, axis=-1, keepdims=True)).astype(np.float32)


def _sigmoid_np(x):
    return (1.0 / (1.0 + np.exp(-x.astype(np.float32)))).astype(np.float32)


def _lstm_lm_np(token_ids, E, Wi, Wh, b):
    x = E[token_ids]                       # [B, T, EMB]
    maskv = token_ids != PAD
    Bt, T = token_ids.shape
    Vu = Wh.shape[0]
    h = np.zeros((Bt, Vu), np.float32)
    c = np.zeros((Bt, Vu), np.float32)
    for t in range(T):
        z = (x[:, t] @ Wi + h @ Wh + b).astype(np.float32)
        i = _sigmoid_np(z[:, :Vu])
        f = _sigmoid_np(z[:, Vu:2 * Vu])
        g = np.tanh(z[:, 2 * Vu:3 * Vu]).astype(np.float32)
        o = _sigmoid_np(z[:, 3 * Vu:])
        c_new = (f * c + i * g).astype(np.float32)
        h_new = (o * np.tanh(c_new)).astype(np.float32)
        m = maskv[:, t][:, None]
        h = np.where(m, h_new, h)
        c = np.where(m, c_new, c)
    return _softmax_np(h)


def _reference_np(input_point, one_softmax, tokens, unfolding_point, E, Wi, Wh,
                  b, curDim, timeStep):
    lat_dim = unfolding_point.shape[-1]
    pad_seq = np.full((input_point.shape[0], 1), PAD, np.int32)
    initial_softmax = _lstm_lm_np(pad_seq, E, Wi, Wh, b)
    if timeStep > 0:
        unfolding_point = input_point
        one_softmax = initial_softmax
    cums = np.cumsum(one_softmax, axis=1, dtype=np.float32)
    cums_excl = (cums - one_softmax).astype(np.float32)
    point = unfolding_point[:, curDim][:, None]
    inside = (cums > point) & (cums_excl <= point)
    token = np.argmax(inside, axis=1)
    low = np.take_along_axis(cums_excl, token[:, None], axis=1)
    size = np.take_along_axis(one_softmax, token[:, None], axis=1)
    new_coord = ((point - low) / size).astype(np.float32)
    unfolding_point = unfolding_point.copy()
    unfolding_point[:, curDim] = new_coord[:, 0]
    tokens = tokens.copy()
    tokens[:, timeStep] = token.astype(tokens.dtype)
    tokens_in = tokens[:, :timeStep + 1].astype(np.int32)
    one_softmax = _lstm_lm_np(tokens_in, E, Wi, Wh, b)
    curDim_new = 0 if curDim + 1 >= lat_dim else curDim + 1
    return (tokens, one_softmax, unfolding_point, np.float32(curDim_new),
            np.int32(timeStep + 1))


# ----------------------------------------------------------------------------
# Entry point
# ----------------------------------------------------------------------------
def kernel(input_point, one_softmax, tokens, unfolding_point, E, Wi, Wh, b,
           curDim, timeStep):
    global LAST_RESULTS
    f32 = np.float32
    input_point = np.ascontiguousarray(np.asarray(input_point, f32))
    one_softmax = np.ascontiguousarray(np.asarray(one_softmax, f32))
    tokens = np.ascontiguousarray(np.asarray(tokens, f32))
    unfolding_point = np.ascontiguousarray(np.asarray(unfolding_point, f32))
    E = np.ascontiguousarray(np.asarray(E, f32))
    Wi = np.ascontiguousarray(np.asarray(Wi, f32))
    Wh = np.asarray(Wh, f32)
    b = np.ascontiguousarray(np.asarray(b, f32))
    cd = int(np.asarray(curDim))
    ts = int(np.asarray(timeStep))

    p = input_point[:, cd] if 0 <= cd < input_point.shape[1] else None
    fast = (
        0 < ts < tokens.shape[1]
        and 0 <= cd < input_point.shape[1]
        and tokens.shape == (B, MAXLEN)
        and input_point.shape == (B, LAT)
        and E.shape == (V, EMB)
        and Wi.shape == (EMB, 4 * V)
        and bool(np.all(tokens[:, :ts] == 0.0))
        and bool(np.all((p >= 0.0) & (p < 1.0)))
    )
    if not fast:
        return _reference_np(input_point, one_softmax, tokens, unfolding_point,
                             E, Wi, Wh, b, cd, ts)

    use_bias = bool(np.any(b != 0.0))
    nc = _get_program(cd, ts, use_bias)

    # per-core packed weights: i/g/o gate columns of this core's hidden slice
    in_maps = []
    for k in range(NCORES):
        lo, hi = k * SH, (k + 1) * SH
        blocks = [Wi[:, lo:hi], Wi[:, 2 * V + lo:2 * V + hi],
                  Wi[:, 3 * V + lo:3 * V + hi]]
        wp = np.concatenate(blocks, axis=1)
        if use_bias:
            brow = np.concatenate([b[lo:hi], b[2 * V + lo:2 * V + hi],
                                   b[3 * V + lo:3 * V + hi]])[None, :]
            wp = np.concatenate([wp, brow], axis=0)
        in_maps.append({
            "ip": input_point,
            "tk": tokens,
            "emb": E,
            "wp": np.ascontiguousarray(wp, f32),
        })

    from concourse import bass_utils
    res = bass_utils.run_bass_kernel_spmd(
        nc, in_maps, core_ids=list(range(NCORES)), trace=TRACE)
    LAST_RESULTS = res
    outs = res.results

    one_softmax_out = np.concatenate(
        [outs[k]["out_shard"] for k in range(NCORES)], axis=1)
    tokens_out = outs[0]["tok_out"]
    unf_out = outs[0]["unf_out"]

    curDim_new = 0 if cd + 1 >= LAT else cd + 1
    return (tokens_out, one_softmax_out, unf_out, np.float32(curDim_new),
            np.int32(ts + 1))
